# revision 38
# baseline (speedup 1.0000x reference)
"""BlipAttention kernel for 8 Trainium2 NeuronCores.

Strategy: data-parallel over batch (16 batches -> 2 per core), no collectives.
Per core: fused QKV projection + 16-head scaled-dot-product attention + output
projection on the PE, bf16 matmuls with fp32 PSUM accumulation.

Final structure (v2 baseline 574us -> 409us), from perfetto trace analysis:
  - combined token axis for the q/k projection: x^T is host-packed
    [D, 1156] (each batch 578 cols, pad col zeroed), so every q/k weight
    m-tile is loaded and LDWEIGHTS'd once for both batches and projection
    matmuls run 512-wide chunks; v/out projections iterate per batch but
    share weight tiles (v) in SBUF.
  - batched softmax normalization (replaces v2's per-head [1,578] DVE
    reciprocal (1.9us each!), gpsimd partition_broadcast, and long PSUM
    holds): each head's raw PV rows + fused denominator row are copied
    PSUM->SBUF bf16 in ~0.5us (PSUM freed immediately), denominators
    collect in a [16,578] tile per batch, one reciprocal_approx_fast per
    batch, a 0/1 head-indicator matmul broadcasts the per-(head,token)
    scale into [128,578] PSUM, and one in-place DVE multiply per apk tile
    half applies it.
  - attention is software-pipelined one token-tile ahead (scores+exp of
    tile tt+1 are emitted before tt's PV matmuls) so the PE FIFO never
    blocks on an in-flight exp; heads of both batches are emitted as soon
    as the q/k m-tile pair covering their features has shipped, hiding all
    exp work under projection matmuls.
  - batch-0's output projection interleaves with batch-1's attention tail;
    PSUM: 3 sc bufs + 2 pv bufs + 3 single-bank accumulators = 8 banks.
  - ~5us of warmup matmuls release the HAM clock gate during input DMA.
"""

import contextlib

import numpy as np
import ml_dtypes

import concourse.tile as tile
from concourse import bacc, mybir
from concourse.bass_utils import run_bass_kernel_spmd

F32 = mybir.dt.float32
BF16 = mybir.dt.bfloat16

N_CORES = 8
B_TOTAL, S, D = 16, 577, 1408
H, HD = 16, 88
SCALE = HD ** -0.5
B = B_TOTAL // N_CORES          # batches per core = 2
T = B * S                       # tokens per core = 1154
SP = S + 1                      # padded per-batch span = 578
TC = B * SP                     # combined token span = 1156
TCPAD = 1160                    # host-padded combined span
KT = D // 128                   # 11 k-tiles over D
VG = HD + 1                     # v group width per head: 88 v cols + 1 one
DEN = HD                        # row of the fused softmax denominator

# per-batch chunks of a 578 span, each within one PSUM bank
CH_S = [(0, 512), (512, 66)]
# combined-token chunks of the 1156 span, each within one PSUM bank
CH_T = [(0, 512), (512, 512), (1024, 132)]
# v projection chunk-groups: (src col base, src width, subs) where each sub
# is (moving col, sub index, width, first head); each sub accumulates in its
# own single-bank PSUM tile
VCH = [
    (880, 528, [(0, 0, 440, 10), (440, 1, 88, 15)]),
    (0, 880, [(0, 0, 440, 0), (440, 1, 440, 5)]),
]


def _tok_tiles():
    out = []
    for tt in range((S + 127) // 128):
        t0 = tt * 128
        out.append((tt, t0, min(128, S - t0)))
    return out


TT = len(_tok_tiles())          # 5


def build_program():
    nc = bacc.Bacc("TRN2", target_bir_lowering=False, debug=False,
                   num_devices=N_CORES)

    x_ap = nc.dram_tensor("xT_bf", [D, TCPAD], BF16, kind="ExternalInput").ap()
    wqk_ap = nc.dram_tensor("wqk_bf", [D, 2 * D], BF16, kind="ExternalInput").ap()
    wv_ap = nc.dram_tensor("wv_bf", [D, D], BF16, kind="ExternalInput").ap()
    wp_ap = nc.dram_tensor("wp_bf", [D, D], BF16, kind="ExternalInput").ap()
    bqk_ap = nc.dram_tensor("b_qk_col", [2 * D, 1], F32, kind="ExternalInput").ap()
    beff_ap = nc.dram_tensor("b_eff_col", [D, 1], F32, kind="ExternalInput").ap()
    eh_ap = nc.dram_tensor("ehead_bf", [H, KT * 128], BF16,
                           kind="ExternalInput").ap()
    outT_ap = nc.dram_tensor("outT", [D, T], BF16, kind="ExternalOutput").ap()

    with tile.TileContext(nc) as tc, contextlib.ExitStack() as ctx:
        p_xT = ctx.enter_context(tc.tile_pool(name="xT", bufs=KT))
        p_vsb = ctx.enter_context(tc.tile_pool(name="vsb", bufs=2 * TT))
        p_qk = ctx.enter_context(tc.tile_pool(name="qk", bufs=28))
        p_expT = ctx.enter_context(tc.tile_pool(name="expT", bufs=6))
        p_atr = ctx.enter_context(tc.tile_pool(name="atr", bufs=4))
        p_apk = ctx.enter_context(tc.tile_pool(name="apk", bufs=KT))
        p_qksb = ctx.enter_context(tc.tile_pool(name="qksb", bufs=4))
        p_den = ctx.enter_context(tc.tile_pool(name="den", bufs=2))
        p_nrm = ctx.enter_context(tc.tile_pool(name="nrm", bufs=3))
        p_esb = ctx.enter_context(tc.tile_pool(name="esb", bufs=1))
        p_wqk = ctx.enter_context(tc.tile_pool(name="wqk", bufs=4))
        p_wp = ctx.enter_context(tc.tile_pool(name="wp", bufs=4))
        p_wv = ctx.enter_context(tc.tile_pool(name="wv", bufs=22))
        p_bias = ctx.enter_context(tc.tile_pool(name="bias", bufs=6))
        p_ot = ctx.enter_context(tc.tile_pool(name="ot", bufs=2))

        p_sc = ctx.enter_context(tc.tile_pool(name="psc", bufs=3, space="PSUM"))
        p_pv = ctx.enter_context(tc.tile_pool(name="ppv", bufs=2, space="PSUM"))
        p_acc = ctx.enter_context(tc.tile_pool(name="pacc", bufs=3, space="PSUM"))

        # combined-token x^T tiles (host pre-transposed/packed)
        xT = [p_xT.tile([128, TCPAD], BF16, tag="xT", name=f"xT{k}")
              for k in range(KT)]
        # v tiles per (batch, token-tile); col 88 of each 89-group memset 1.0
        vsb = [[p_vsb.tile([128, H * VG], BF16, tag="vsb", name=f"vsb{b}_{tt}")
                for tt in range(TT)] for b in range(B)]
        # combined-token feature-major attention output (raw, then normalized
        # in place); batch b occupies cols [b*SP, b*SP+SP)
        apk = [p_apk.tile([128, TC], BF16, tag="apk", name=f"apk{k}")
               for k in range(KT)]
        den16 = [p_den.tile([H, SP], BF16, tag="den", name=f"den{b}")
                 for b in range(B)]
        esb = p_esb.tile([H, KT * 128], BF16, tag="esb", name="esb")
        qh = [[None] * H for _ in range(B)]
        kh = [[None] * H for _ in range(B)]

        def emit_warmup():
            # ~5us of back-to-back dummy matmuls so the HAM clock gate is
            # released (K=8/8) by the time real operands arrive
            wsrc = p_qksb.tile([128, TC], BF16, tag="qksb", name="warm_src")
            nc.gpsimd.memset(wsrc[:, 0:128], 0.0)
            wps = p_acc.tile([128, 512], F32, tag="acc", name="warm_ps")
            for _ in range(56):
                nc.tensor.matmul(wps[:, 0:128], wsrc[:, 0:128],
                                 wsrc[:, 0:128], start=True, stop=True)

        def emit_loads():
            engs = [nc.sync, nc.scalar, nc.gpsimd]
            for k in range(KT):
                engs[k % 3].dma_start(xT[k][:],
                                      x_ap[k * 128:(k + 1) * 128, :])
            for b in range(B):
                for tt in range(TT):
                    ones = vsb[b][tt][:].rearrange("p (h g) -> p h g",
                                                   g=VG)[:, :, DEN:DEN + 1]
                    nc.gpsimd.memset(ones, 1.0)

        def prefetch_wv():
            wvt = [[None] * KT for _ in VCH]
            for cg, (s0, sw, subs) in enumerate(VCH):
                for k in range(KT):
                    wv_t = p_wv.tile([128, 880], BF16, tag="wv",
                                     name=f"wv{cg}_{k}")
                    nc.gpsimd.dma_start(
                        wv_t[:, 0:sw],
                        wv_ap[k * 128:(k + 1) * 128, s0:s0 + sw])
                    wvt[cg][k] = wv_t
            return wvt

        def emit_vproj(b, wvt_all):
            """generator: one unit per (chunk-group, token-tile) for one
            batch; weight tiles are shared across batches"""
            for cg, (s0, sw, subs) in enumerate(VCH):
                wvt = wvt_all[cg]
                if True:
                    for tt, t0, ts in _tok_tiles():
                        acc = [p_acc.tile([128, 512], F32, tag="acc",
                                          name=f"vacc{si}")
                               for si in range(2)]
                        for ki in range(KT):
                            for (m0, si, w, h0) in subs:
                                nc.tensor.matmul(
                                    acc[si][0:ts, 0:w],
                                    xT[ki][:, b * SP + t0: b * SP + t0 + ts],
                                    wvt[ki][:, m0:m0 + w],
                                    start=(ki == 0), stop=(ki == KT - 1))
                        for (m0, si, w, h0) in subs:
                            nh = w // HD
                            dst = vsb[b][tt][0:ts, :].rearrange(
                                "p (h g) -> p h g", g=VG)[:, h0:h0 + nh, 0:HD]
                            src = acc[si][0:ts, 0:w].rearrange(
                                "p (h g) -> p h g", g=HD)
                            nc.vector.tensor_copy(dst, src)
                        yield

        def _ship_heads(which, fidx, qksb):
            f_lo, f_hi = fidx * 128, fidx * 128 + 128
            for b in range(B):
                dst_l = qh[b] if which == 0 else kh[b]
                for h in range(f_lo // HD, min(H, (f_hi + HD - 1) // HD)):
                    s0 = max(f_lo, h * HD)
                    s1 = min(f_hi, (h + 1) * HD)
                    if s1 <= s0:
                        continue
                    if dst_l[h] is None:
                        dst_l[h] = p_qk.tile([HD, SP], BF16, tag="qk",
                                             name=f"qk{b}_{which}_{h}")
                    r0 = s0 - h * HD
                    nc.gpsimd.dma_start(
                        dst_l[h][r0: r0 + (s1 - s0), :],
                        qksb[s0 - f_lo: s1 - f_lo, b * SP: b * SP + SP])

        def emit_qk_unit(m):
            """one combined-token q/k m-tile (0..10 q, 11..21 k)"""
            col = m * 128
            wqt = p_wqk.tile([128, KT * 128], BF16, tag="wqk",
                             name=f"wq{m}")
            nc.sync.dma_start(
                wqt[:].rearrange("p (k c) -> p k c", k=KT),
                wqk_ap[:, col: col + 128].rearrange("(k p) c -> p k c",
                                                    p=128))
            bq = p_bias.tile([128, 1], F32, tag="bias", name="bq")
            nc.sync.dma_start(bq[:], bqk_ap[col: col + 128, :])
            acc = [p_acc.tile([128, 512], F32, tag="acc", name=f"qkacc{ci}")
                   for ci in range(3)]
            for ki in range(KT):
                for ci, (lc, w) in enumerate(CH_T):
                    nc.tensor.matmul(acc[ci][0:128, 0:w],
                                     wqt[:, ki * 128:(ki + 1) * 128],
                                     xT[ki][:, lc:lc + w],
                                     start=(ki == 0), stop=(ki == KT - 1))
            qksb = p_qksb.tile([128, TC], BF16, tag="qksb")
            for ci, (lc, w) in enumerate(CH_T):
                nc.vector.tensor_scalar_add(qksb[:, lc:lc + w],
                                            acc[ci][0:128, 0:w], bq[:])
            which, fidx = (0, m) if m < KT else (1, m - KT)
            _ship_heads(which, fidx, qksb)

        def emit_att_head(b, h):
            """scores/exp/PV for one head; raw PV rows + denominator row are
            copied out bf16 immediately so the pv PSUM tiles free fast"""
            pv = [p_pv.tile([VG, 512], F32, tag="pv", name=f"pv{si}")
                  for si in range(2)]

            def stage(tt, t0, ts):
                ets = []
                for si, (lc, w) in enumerate(CH_S):
                    sc = p_sc.tile([128, 512], F32, tag="sc", name="sc")
                    nc.tensor.matmul(sc[0:ts, 0:w],
                                     kh[b][h][:, t0:t0 + ts],
                                     qh[b][h][:, lc:lc + w],
                                     start=True, stop=True)
                    et = p_expT.tile([128, 512], BF16, tag="expT")
                    nc.scalar.activation(et[0:ts, 0:w], sc[0:ts, 0:w],
                                         mybir.ActivationFunctionType.Exp,
                                         scale=SCALE)
                    ets.append((si, w, et))
                return ets

            def pv_mms(tt, ts, ets):
                for si, w, et in ets:
                    nc.tensor.matmul(pv[si][0:VG, 0:w],
                                     vsb[b][tt][0:ts, h * VG:(h + 1) * VG],
                                     et[0:ts, 0:w],
                                     start=(tt == 0), stop=(tt == TT - 1))

            prev = None
            for tt, t0, ts in _tok_tiles():
                ets = stage(tt, t0, ts)
                if prev is not None:
                    pv_mms(*prev)
                prev = (tt, ts, ets)
            pv_mms(*prev)
            atr = p_atr.tile([VG, SP], BF16, tag="atr", name="atr")
            nc.vector.tensor_copy(atr[0:VG, 0:512], pv[0][0:VG, 0:512])
            nc.vector.tensor_copy(atr[0:VG, 512:SP], pv[1][0:VG, 0:66])
            # raw (unnormalized) v rows into the feature-major apk tiles
            f0 = h * HD
            k0, r0 = f0 // 128, f0 % 128
            n0 = min(HD, 128 - r0)
            nc.sync.dma_start(apk[k0][r0: r0 + n0, b * SP: b * SP + SP],
                                atr[0:n0, :])
            if n0 < HD:
                nc.sync.dma_start(apk[k0 + 1][0: HD - n0,
                                              b * SP: b * SP + SP],
                                  atr[n0:HD, :])
            # denominator row into the batch den tile
            nc.sync.dma_start(den16[b][h:h + 1, :], atr[DEN:DEN + 1, :])

        def emit_norm(b):
            """batched softmax normalization for one batch half: reciprocal
            of the 16 denominator rows, head-indicator matmul broadcast,
            in-place multiplies on the apk column halves (split between the
            vector and gpsimd engines)"""
            d16f = p_nrm.tile([H, SP], F32, tag="nrm", name="d16f")
            nc.vector.tensor_copy(d16f[:], den16[b][:])
            r16f = p_nrm.tile([H, SP], F32, tag="nrm", name="r16f")
            nc.vector.reciprocal_approx_fast(r16f[:], d16f[:])
            r16 = p_nrm.tile([H, SP], BF16, tag="nrm", name="r16")
            nc.vector.tensor_copy(r16[:], r16f[:])
            for k in range(KT):
                eng = nc.vector
                for ci, (lc, w) in enumerate(CH_S):
                    recb = p_acc.tile([128, 512], F32, tag="acc",
                                      name=f"recb{ci}")
                    nc.tensor.matmul(recb[0:128, 0:w],
                                     esb[:, k * 128:(k + 1) * 128],
                                     r16[:, lc:lc + w],
                                     start=True, stop=True)
                    eng.tensor_mul(
                        apk[k][:, b * SP + lc: b * SP + lc + w],
                        apk[k][:, b * SP + lc: b * SP + lc + w],
                        recb[0:128, 0:w])

        def emit_op_unit(b, oc):
            """one output-projection feature tile for one batch half"""
            wpt = p_wp.tile([128, KT * 128], BF16, tag="wp",
                            name=f"wp{b}_{oc}")
            nc.sync.dma_start(
                wpt[:].rearrange("p (k c) -> p k c", k=KT),
                wp_ap[:, oc * 128:(oc + 1) * 128].rearrange(
                    "(k p) c -> p k c", p=128))
            be = p_bias.tile([128, 1], F32, tag="bias", name="be")
            nc.sync.dma_start(be[:], beff_ap[oc * 128:(oc + 1) * 128, :])
            ot = p_ot.tile([128, SP], BF16, tag="ot")
            acc = [p_acc.tile([128, 512], F32, tag="acc", name=f"oacc{ci}")
                   for ci in range(2)]
            for ki in range(KT):
                for ci, (lc, w) in enumerate(CH_S):
                    nc.tensor.matmul(acc[ci][0:128, 0:w],
                                     wpt[:, ki * 128:(ki + 1) * 128],
                                     apk[ki][:, b * SP + lc: b * SP + lc + w],
                                     start=(ki == 0), stop=(ki == KT - 1))
            for ci, (lc, w) in enumerate(CH_S):
                nc.vector.tensor_scalar_add(ot[:, lc:lc + w],
                                            acc[ci][0:128, 0:w], be[:])
            nc.sync.dma_start(outT_ap[oc * 128:(oc + 1) * 128,
                                      b * S:(b + 1) * S],
                              ot[:, 0:S])

        # ---- emission schedule ----
        # q/k m-tiles in (q_j, k_j) pairs; head h's attention is emitted as
        # soon as pairs 0..j cover its features (batch 1 one pair behind so
        # batch 0's normalization overlaps batch 1's tail heads).
        def head_ready(j):
            if j < 0:
                return 0
            return min(H, ((j + 1) * 128) // HD)

        emit_warmup()
        nc.sync.dma_start(esb[:], eh_ap[:, :])
        emit_loads()
        wvt = prefetch_wv()
        for _ in emit_vproj(0, wvt):
            pass
        for _ in emit_vproj(1, wvt):
            pass

        e0 = e1 = 0
        for j in range(KT):
            emit_qk_unit(j)
            emit_qk_unit(KT + j)
            while e0 < head_ready(j):
                emit_att_head(0, e0)
                e0 += 1
            while e1 < head_ready(j):
                emit_att_head(1, e1)
                e1 += 1
        emit_norm(0)
        for oc in range(KT):
            if e1 < H:
                emit_att_head(1, e1)
                e1 += 1
            emit_op_unit(0, oc)
        emit_norm(1)
        for oc in range(KT):
            emit_op_unit(1, oc)

    nc.compile()
    return nc


_NC_CACHE = None


def _get_nc():
    global _NC_CACHE
    if _NC_CACHE is None:
        _NC_CACHE = build_program()
    return _NC_CACHE


def make_in_maps(hidden_states, w_qkv, b_qkv, w_proj, b_proj):
    hidden_states = np.asarray(hidden_states, dtype=np.float32)
    w_qkv = np.ascontiguousarray(np.asarray(w_qkv, dtype=np.float32))
    b_qkv = np.asarray(b_qkv, dtype=np.float32)
    w_proj = np.ascontiguousarray(np.asarray(w_proj, dtype=np.float32))
    b_proj = np.asarray(b_proj, dtype=np.float32)

    wqk_bf = w_qkv[:, : 2 * D].astype(ml_dtypes.bfloat16)
    wv_bf = np.ascontiguousarray(w_qkv[:, 2 * D:]).astype(ml_dtypes.bfloat16)
    wp_bf = w_proj.astype(ml_dtypes.bfloat16)
    bqk_col = b_qkv[: 2 * D].reshape(2 * D, 1).copy()
    # v-bias folded through the output projection: probs rows sum to 1
    b_eff = (b_qkv[2 * D:] @ w_proj + b_proj).reshape(D, 1).astype(np.float32)

    # head-indicator matrix: ehead[h, k*128 + p] = 1 iff feature 128k+p
    # belongs to head h
    feat = np.arange(KT * 128)
    ehead = (feat[None, :] // HD == np.arange(H)[:, None])
    ehead_bf = ehead.astype(ml_dtypes.bfloat16)

    hs_bf = hidden_states.astype(ml_dtypes.bfloat16)
    in_maps = []
    for c in range(N_CORES):
        xb = np.zeros((D, TCPAD), dtype=ml_dtypes.bfloat16)
        for b in range(B):
            xb[:, b * SP: b * SP + S] = hs_bf[c * B + b].T
        in_maps.append({
            "xT_bf": xb,
            "wqk_bf": wqk_bf,
            "wv_bf": wv_bf,
            "wp_bf": wp_bf,
            "b_qk_col": bqk_col,
            "b_eff_col": b_eff,
            "ehead_bf": ehead_bf,
        })
    return in_maps


def kernel(hidden_states, w_qkv, b_qkv, w_proj, b_proj):
    nc = _get_nc()
    in_maps = make_in_maps(hidden_states, w_qkv, b_qkv, w_proj, b_proj)
    res = run_bass_kernel_spmd(nc, in_maps, list(range(N_CORES)))
    outs = []
    for c in range(N_CORES):
        oT = np.asarray(res.results[c]["outT"], dtype=np.float32)  # [D, T]
        outs.append(oT.T.reshape(B, S, D))
    return np.concatenate(outs, axis=0).astype(np.float32)


if __name__ == "__main__":
    rng = np.random.default_rng(0)
    hs = rng.standard_normal((B_TOTAL, S, D), dtype=np.float32)
    wq = rng.standard_normal((D, 3 * D), dtype=np.float32) * D ** -0.5
    bq = rng.standard_normal(3 * D).astype(np.float32) * 0.02
    wp = rng.standard_normal((D, D), dtype=np.float32) * D ** -0.5
    bp = rng.standard_normal(D).astype(np.float32) * 0.02
    o = kernel(hidden_states=hs, w_qkv=wq, b_qkv=bq, w_proj=wp, b_proj=bp)
    print(o.shape, o.dtype)
